# revision 1
# baseline (speedup 1.0000x reference)
"""Causal self-attention (B=4, T=2048, C=1024, H=16) on 8 TRN2 NeuronCores.

Sharding: core c -> (batch b = c//2, head-group g = c%2). Each core computes
QKV for its 8 heads of one batch, causal attention, and a partial output
projection (its heads' slice of W_proj). The pairwise reduction over head
groups (the "all-reduce after c_proj") plus b_proj is done on host at
gather time.

On-core layout: everything transposed so the model/head dim lives on SBUF
partitions:
  xT       [C, T]      (host pre-transposes x[b])
  QK^T     [1024, T]   rows 0:512 = Q^T (8 heads x 64), 512:1024 = K^T
  V        [T, 512]    + a ones column per head -> fused softmax denominator
  S^T      [k, q] blocks of [128, 512]; exp on ScalarE straight out of PSUM;
           causal handled by skipping fully-masked blocks and multiplying
           diagonal blocks with 0/1 masks
  y^T      [65, 512] PSUM accum per (head, q-chunk): rows 0:64 = V^T @ P^T,
           row 64 = softmax denominator (ones column)
  out      y_part^T [C, T] = Wp_slice^T-contract; host transposes + sums.
Matmuls run as float32r (full PE rate; fp32 storage, reduced mantissa in PE).
"""

import sys

for _p in ("/opt/trn_rl_repo", "/root/.axon_site/_ro/trn_rl_repo"):
    if _p not in sys.path:
        sys.path.insert(0, _p)

import numpy as np

import concourse.bass as bass
import concourse.mybir as mybir
import concourse.tile as tile
from concourse.bass import ts
from concourse.bass_utils import run_bass_kernel_spmd

B, T, C, H, HD = 4, 2048, 1024, 16, 64
NH = 8           # heads per core
P = 128
QC = 512         # q-chunk width
NQC = T // QC    # 4
NKB = T // P     # 16 k-blocks
KO = C // P      # 8 contraction tiles for the C-dim
F32 = mybir.dt.float32
F32R = mybir.dt.float32r


def _r(ap):
    return ap


def build_nc():
    nc = bass.Bass()

    xT = nc.dram_tensor("xT", [C, T], F32R, kind="ExternalInput")
    Wqk = nc.dram_tensor("Wqk", [C, 2 * NH * HD], F32R, kind="ExternalInput")
    Wv = nc.dram_tensor("Wv", [C, NH * HD], F32R, kind="ExternalInput")
    Wp = nc.dram_tensor("Wp", [NH * HD, C], F32R, kind="ExternalInput")
    bqk = nc.dram_tensor("bqk", [P, 2 * NH * HD // P], F32, kind="ExternalInput")
    bv = nc.dram_tensor("bv", [NH * HD], F32, kind="ExternalInput")
    masks = nc.dram_tensor("masks", [QC // P, P, QC], F32R, kind="ExternalInput")
    yT = nc.dram_tensor("yT", [C, T], F32, kind="ExternalOutput")

    xT_t = xT[:].rearrange("(ko p) t -> p ko t", p=P)        # [128, 8, T]
    yT_t = yT[:].rearrange("(mo p) t -> p mo t", p=P)        # [128, 8, T]
    Wqk_t = Wqk[:].rearrange("(ko p) n -> p ko n", p=P)      # [128, 8, 1024]
    Wv_t = Wv[:].rearrange("(ko p) n -> p ko n", p=P)        # [128, 8, 512]
    Wp_t = Wp[:].rearrange("(ko p) n -> p ko n", p=P)        # [128, 4, 1024]

    with tile.TileContext(nc) as tc:
        with (
            tc.tile_pool(name="consts", bufs=1) as consts,
            tc.tile_pool(name="persist", bufs=1) as persist,
            tc.tile_pool(name="w1", bufs=1) as w1pool,
            tc.tile_pool(name="xt", bufs=1) as xtpool,
            tc.tile_pool(name="qt", bufs=2) as qtpool,
            tc.tile_pool(name="yt", bufs=2) as ytpool,
            tc.tile_pool(name="pt", bufs=3) as ptpool,
            tc.tile_pool(name="recb", bufs=2) as rbpool,
            tc.tile_pool(name="rec", bufs=1) as rpool,
            tc.tile_pool(name="st", bufs=2) as stpool,
            tc.tile_pool(name="ps_s1", bufs=2, space="PSUM") as ps_s1,
            tc.tile_pool(name="ps_sc", bufs=2, space="PSUM") as ps_sc,
            tc.tile_pool(name="ps_rb", bufs=1, space="PSUM") as ps_rb,
            tc.tile_pool(name="ps_y", bufs=2, space="PSUM") as ps_y,
            tc.tile_pool(name="ps_p", bufs=1, space="PSUM") as ps_p,
        ):
            # ---- constants ----
            bqk_sb = consts.tile([P, 2 * NH * HD // P], F32)      # [128, 8]
            nc.sync.dma_start(bqk_sb[:], bqk[:])
            bv_sb = consts.tile([P, NH * HD], F32)                # [128, 512]
            nc.sync.dma_start(bv_sb[:], bass.AP(bv, 0, [[0, P], [1, NH * HD]]))
            masks_sb = consts.tile([P, QC // P, QC], F32R)         # [128, 4, 512]
            nc.sync.dma_start(masks_sb[:], masks[:].rearrange("d p q -> p d q"))
            wp_sb = consts.tile([P, NH * HD // P, C], F32R)        # [128, 4, 1024]
            nc.sync.dma_start(wp_sb[:], Wp_t[:])

            ones_row = consts.tile([1, HD], F32R)
            nc.vector.memset(ones_row[:].bitcast(F32), 1.0)

            # ---- persistent activations ----
            kt_sb = persist.tile([P, NH * HD // P, T], F32R)       # [128, 4, 2048]
            vex_sb = persist.tile([P, NKB, NH, HD + 1], F32R)      # [128,16,8,65]
            nc.vector.memset(vex_sb[:, :, :, HD:].bitcast(F32), 1.0)

            # ---- stage-1 weights ----
            wqk_sb = w1pool.tile([P, KO, 2 * NH * HD], F32R)       # 4MB
            nc.sync.dma_start(wqk_sb[:], Wqk_t[:])
            wv_sb = w1pool.tile([P, KO, NH * HD], F32R)            # 2MB
            nc.sync.dma_start(wv_sb[:], Wv_t[:])

            for tc_i in range(NQC):  # T chunk of 512
                # ---------- stage 1 for this T-chunk ----------
                xt = xtpool.tile([P, KO, QC], F32R)
                nc.sync.dma_start(xt[:], xT_t[:, :, ts(tc_i, QC)])

                # QK^T rows: m 0..3 -> Q^T (transient, this chunk only),
                # m 4..7 -> K^T (persistent)
                qt = qtpool.tile([P, NH * HD // P, QC], F32R)
                for m in range(2 * NH * HD // P):  # 8
                    ps = ps_s1.tile([P, QC], F32, tag="s1")
                    for k in range(KO):
                        nc.tensor.matmul(
                            ps[:],
                            _r(wqk_sb[:, k, ts(m, P)]),
                            _r(xt[:, k, :]),
                            start=(k == 0),
                            stop=(k == KO - 1),
                        )
                    if m < NH * HD // P:
                        dst = qt[:, m, :]
                    else:
                        dst = kt_sb[:, m - NH * HD // P, ts(tc_i, QC)]
                    nc.vector.tensor_scalar_add(dst, ps[:], bqk_sb[:, m : m + 1])

                # V rows for the 4 k-blocks of this T-chunk
                for t4 in range(QC // P):
                    kb = tc_i * (QC // P) + t4
                    psv = ps_s1.tile([P, NH * HD], F32, tag="s1")
                    for k in range(KO):
                        nc.tensor.matmul(
                            psv[:],
                            _r(xt[:, k, ts(t4, P)]),
                            _r(wv_sb[:, k, :]),
                            start=(k == 0),
                            stop=(k == KO - 1),
                        )
                    nc.vector.tensor_add(
                        vex_sb[:, kb, :, :HD],
                        psv[:].rearrange("p (h d) -> p h d", h=NH),
                        bv_sb[:].rearrange("p (h d) -> p h d", h=NH),
                    )

                # ---------- attention + proj for q-chunk == this T-chunk ----------
                qc = tc_i
                ytq = ytpool.tile([P, NH * HD // P, QC], F32R)     # [128, 4, 512]
                for h in range(NH):
                    pb = (h % 2) * HD          # partition base for this head
                    mq = h // 2                # Q^T m-tile (in qt)
                    mk = h // 2                # K^T m-tile (in kt_sb)
                    nkb = (qc + 1) * (QC // P)
                    yac = ps_y.tile([HD + 1, QC], F32)
                    for kb in range(nkb):
                        sps = ps_sc.tile([P, QC], F32)
                        nc.tensor.matmul(
                            sps[:],
                            _r(kt_sb[pb : pb + HD, mk, ts(kb, P)]),
                            _r(qt[pb : pb + HD, mq, :]),
                            start=True,
                            stop=True,
                        )
                        pt = ptpool.tile([P, QC], F32R)
                        nc.scalar.activation(
                            pt[:], sps[:], mybir.ActivationFunctionType.Exp,
                            scale=1.0 / np.sqrt(HD),
                        )
                        d = kb - qc * (QC // P)
                        if d >= 0:  # diagonal block: 0/1 mask
                            nc.vector.tensor_mul(pt[:], pt[:], masks_sb[:, d, :])
                        nc.tensor.matmul(
                            yac[:],
                            _r(vex_sb[:, kb, h, :]),
                            _r(pt[:]),
                            start=(kb == 0),
                            stop=(kb == nkb - 1),
                        )
                    rec = rpool.tile([1, QC], F32R)
                    nc.vector.reciprocal(rec[:].bitcast(F32), yac[HD : HD + 1, :])
                    rec_r = rpool.tile([1, QC], F32R, tag="rec_r")
                    nc.vector.tensor_copy(rec_r[:], rec[:].bitcast(F32))
                    recb_ps = ps_rb.tile([HD, QC], F32)
                    nc.tensor.matmul(recb_ps[:], ones_row[:], rec_r[:], start=True, stop=True)
                    recb = rbpool.tile([HD, QC], F32)
                    nc.vector.tensor_copy(recb[:], recb_ps[:])
                    nc.vector.tensor_mul(
                        ytq[pb : pb + HD, h // 2, :], yac[:HD, :], recb[:]
                    )

                # proj: y_part^T[:, qc] = Wp_slice.T-contract @ ytq
                for m in range(C // P):  # 8
                    pp = ps_p.tile([P, QC], F32)
                    for kk in range(NH * HD // P):  # 4
                        nc.tensor.matmul(
                            pp[:],
                            _r(wp_sb[:, kk, ts(m, P)]),
                            _r(ytq[:, kk, :]),
                            start=(kk == 0),
                            stop=(kk == NH * HD // P - 1),
                        )
                    st = stpool.tile([P, QC], F32)
                    nc.vector.tensor_copy(st[:], pp[:])
                    nc.sync.dma_start(yT_t[:, m, ts(qc, QC)], st[:])

    return nc


def legalize_waits(nc):
    """This walrus build accepts at most 1 sync wait per instruction (0 for
    self-loading fp32/fp32r Matmult, whose LW slot takes none). Move excess
    waits onto preceding same-engine NoOps; engines execute in order so the
    guarantee is identical."""
    n = 0
    for blk in nc.m.functions[0].blocks:
        new = []
        for inst in blk.instructions:
            si = inst.sync_info
            waits = list(si.on_wait) if si is not None and si.on_wait else []
            lim = 0 if inst.opcode in ("Matmult", "Ldweights") else 1
            if len(waits) > lim:
                keep = waits[len(waits) - lim:] if lim else []
                for w in waits[: len(waits) - lim]:
                    n += 1
                    new.append(mybir.InstNoOp(
                        name=f"I-wfix{n}", engine=inst.engine, ins=[], outs=[],
                        sync_info=mybir.SyncInfo(on_wait=[w], on_update=[]),
                    ))
                inst.sync_info = mybir.SyncInfo(
                    on_wait=keep,
                    on_update=list(si.on_update) if si.on_update else [],
                )
            new.append(inst)
        blk.instructions = new
    return n


def _host_inputs(x, W_attn, b_attn, W_proj):
    """Build the 8 per-core input maps."""
    # causal 0/1 masks for the 4 diagonal-crossing block offsets
    kl = np.arange(P)[:, None]
    ql = np.arange(QC)[None, :]
    masks = np.stack(
        [(ql >= kl + d * P).astype(np.float32) for d in range(QC // P)]
    )  # [4, 128, 512]

    in_maps = []
    for core in range(8):
        b, g = core // 2, core % 2
        qs = slice(g * NH * HD, (g + 1) * NH * HD)
        ks = slice(C + g * NH * HD, C + (g + 1) * NH * HD)
        vs = slice(2 * C + g * NH * HD, 2 * C + (g + 1) * NH * HD)
        wqk = np.ascontiguousarray(
            np.concatenate([W_attn[:, qs], W_attn[:, ks]], axis=1)
        )
        bqk = (
            np.concatenate([b_attn[qs], b_attn[ks]])
            .reshape(2 * NH * HD // P, P)
            .T.copy()
        )
        in_maps.append(
            {
                "xT": np.ascontiguousarray(x[b].T),
                "Wqk": wqk,
                "Wv": np.ascontiguousarray(W_attn[:, vs]),
                "Wp": np.ascontiguousarray(W_proj[g * NH * HD : (g + 1) * NH * HD]),
                "bqk": np.ascontiguousarray(bqk),
                "bv": np.ascontiguousarray(b_attn[vs]),
                "masks": masks,
            }
        )
    return in_maps


def run(x, W_attn, b_attn, W_proj, b_proj, trace=False):
    """Returns (y, BassKernelResults)."""
    x = np.asarray(x, dtype=np.float32)
    W_attn = np.asarray(W_attn, dtype=np.float32)
    b_attn = np.asarray(b_attn, dtype=np.float32)
    W_proj = np.asarray(W_proj, dtype=np.float32)
    b_proj = np.asarray(b_proj, dtype=np.float32)

    nc = build_nc()
    legalize_waits(nc)
    in_maps = _host_inputs(x, W_attn, b_attn, W_proj)
    res = run_bass_kernel_spmd(nc, in_maps, list(range(8)), trace=trace)

    y = np.empty((B, T, C), dtype=np.float32)
    for b in range(B):
        acc = res.results[2 * b]["yT"] + res.results[2 * b + 1]["yT"]
        y[b] = acc.T + b_proj
    return y, res


def kernel(x, W_attn, b_attn, W_proj, b_proj):
    y, _ = run(x, W_attn, b_attn, W_proj, b_proj)
    return y



# revision 10
# speedup vs baseline: 1.7462x; 1.7462x over previous
"""Causal self-attention (B=4, T=2048, C=1024, H=16) on 8 TRN2 NeuronCores.

Sharding: core c -> (batch b = c//2, head-group g = c%2). Each core computes
QKV for its 8 heads of one batch, causal attention, and a partial output
projection (its heads' slice of W_proj). The pairwise reduction over head
groups (the "all-reduce after c_proj") plus b_proj is done on host at
gather time.

On-core layout (v2 — PE kept at full clock):
  xT        [C, T]     (host pre-transposes x[b]); stage-1 QKV in fp32r
  qt        [128, 4, 512] bf16 — Q^T per chunk, head pair on partitions
  kt_pad    [128, 8, T] bf16 — per-head K^T, zero-padded to the full 128
            contraction partitions so S matmuls are full unmasked
            128x128xW (row_grp-masked matmuls don't register as PE
            activity in the HAM clock gate and throttle the PE to 1.2GHz)
  S         [128 k-rows, W q] psum; W trimmed to the causal width
            (512-128d on diagonal blocks); exp on ScalarE -> pt bf16;
            the single 128x128 (j>=p) boundary mask applied on GpSimd
  y         [65, 512] psum per (head, chunk): rows 0:64 = V^T P, row 64 =
            softmax denominator via a bf16 ones column in vex
  recip     reciprocal_approx_fast (DVE custom op, ~5x cheaper), then one
            full 128-contraction broadcast matmul per head PAIR
  proj      Wp bf16; psum -> SBUF copy -> DMA out as y_part^T [C, T];
            host transposes + sums the two head groups + b_proj.
"""

import sys

for _p in ("/opt/trn_rl_repo", "/root/.axon_site/_ro/trn_rl_repo"):
    if _p not in sys.path:
        sys.path.insert(0, _p)

import ml_dtypes
import numpy as np

import concourse.bass as bass
import concourse.mybir as mybir
import concourse.tile as tile
from concourse.bass import ts
from concourse.bass_utils import run_bass_kernel_spmd

B, T, C, H, HD = 4, 2048, 1024, 16, 64
NH = 8           # heads per core
P = 128
QC = 512         # q-chunk width
NQC = T // QC    # 4
NKB = T // P     # 16 k-blocks
KO = C // P      # 8 contraction tiles for the C-dim
F32 = mybir.dt.float32
F32R = mybir.dt.float32r
BF16 = mybir.dt.bfloat16


def build_nc():
    nc = bass.Bass()

    xT = nc.dram_tensor("xT", [C, T], F32R, kind="ExternalInput")
    Wqk = nc.dram_tensor("Wqk", [C, 2 * NH * HD], F32R, kind="ExternalInput")
    Wv = nc.dram_tensor("Wv", [C, NH * HD], F32R, kind="ExternalInput")
    Wp = nc.dram_tensor("Wp", [NH * HD, C], BF16, kind="ExternalInput")
    bqk = nc.dram_tensor("bqk", [P, 2 * NH * HD // P], F32, kind="ExternalInput")
    bv = nc.dram_tensor("bv", [NH * HD], F32, kind="ExternalInput")
    mask = nc.dram_tensor("mask", [P, P], BF16, kind="ExternalInput")
    yT = nc.dram_tensor("yT", [C, T], F32, kind="ExternalOutput")

    xT_t = xT[:].rearrange("(ko p) t -> p ko t", p=P)        # [128, 8, T]
    yT_t = yT[:].rearrange("(mo p) t -> p mo t", p=P)        # [128, 8, T]
    Wqk_t = Wqk[:].rearrange("(ko p) n -> p ko n", p=P)      # [128, 8, 1024]
    Wv_t = Wv[:].rearrange("(ko p) n -> p ko n", p=P)        # [128, 8, 512]
    Wp_t = Wp[:].rearrange("(ko p) n -> p ko n", p=P)        # [128, 4, 1024]

    with tile.TileContext(nc) as tc:
        with (
            tc.tile_pool(name="consts", bufs=1) as consts,
            tc.tile_pool(name="persist", bufs=1) as persist,
            tc.tile_pool(name="w1", bufs=1) as w1pool,
            tc.tile_pool(name="xt", bufs=2) as xtpool,
            tc.tile_pool(name="qt", bufs=2) as qtpool,
            tc.tile_pool(name="yt", bufs=2) as ytpool,
            tc.tile_pool(name="pt", bufs=3) as ptpool,
            tc.tile_pool(name="recb", bufs=2) as rbpool,
            tc.tile_pool(name="ln", bufs=2) as lnpool,
            tc.tile_pool(name="st", bufs=2) as stpool,
            tc.tile_pool(name="ps_s1", bufs=2, space="PSUM") as ps_s1,
            tc.tile_pool(name="ps_sc", bufs=2, space="PSUM") as ps_sc,
            tc.tile_pool(name="ps_y", bufs=2, space="PSUM") as ps_y,
            tc.tile_pool(name="ps_p", bufs=2, space="PSUM") as ps_p,
        ):
            # ---- constants ----
            bqk_sb = consts.tile([P, 2 * NH * HD // P], F32)      # [128, 8]
            nc.sync.dma_start(bqk_sb[:], bqk[:])
            bv_sb = consts.tile([P, NH * HD], F32)                # [128, 512]
            nc.sync.dma_start(bv_sb[:], bass.AP(bv, 0, [[0, P], [1, NH * HD]]))
            mask_sb = consts.tile([P, P], BF16)                    # j >= p
            nc.sync.dma_start(mask_sb[:], mask[:])
            wp_sb = consts.tile([P, NH * HD // P, C], BF16)        # [128, 4, 1024]
            nc.sync.dma_start(wp_sb[:], Wp_t[:])

            # recb broadcast operands: ones2 row 0 selects out cols 0:64
            # (head A), row 64 cols 64:128 (head B); other rows are zero, as
            # are reczero's other moving rows, so the matmul only sees the
            # two recs. (Row 64, not 1: engine APs need base partition
            # 0/32/64/96.)
            ones2 = consts.tile([P, P], F32)
            nc.vector.memset(ones2[:], 0.0)
            nc.vector.memset(ones2[0:1, 0:HD], 1.0)
            nc.vector.memset(ones2[HD : HD + 1, HD : 2 * HD], 1.0)
            reczero = consts.tile([P, QC], F32)
            nc.vector.memset(reczero[:], 0.0)

            # ---- persistent activations ----
            kt_pad = persist.tile([P, NH, T], BF16)                # 4MB
            nc.vector.memset(kt_pad[HD:P, 0:NH:2, :], 0.0)
            nc.vector.memset(kt_pad[0:HD, 1:NH:2, :], 0.0)
            vex = persist.tile([P, NKB, NH, HD + 1], BF16)         # 2.1MB
            nc.vector.memset(vex[:, :, :, HD:], 1.0)

            # ---- stage-1 weights ----
            wqk_sb = w1pool.tile([P, KO, 2 * NH * HD], F32R)       # 4MB
            nc.sync.dma_start(wqk_sb[:], Wqk_t[:])
            wv_sb = w1pool.tile([P, KO, NH * HD], F32R)            # 2MB
            nc.sync.dma_start(wv_sb[:], Wv_t[:])

            for tc_i in range(NQC):  # T chunk of 512
                # ---------- stage 1 for this T-chunk ----------
                xt = xtpool.tile([P, KO, QC], F32R)
                nc.sync.dma_start(xt[:], xT_t[:, :, ts(tc_i, QC)])

                qt = qtpool.tile([P, NH * HD // P, QC], BF16)
                for m in range(2 * NH * HD // P):  # 8: m<4 Q, m>=4 K
                    ps = ps_s1.tile([P, QC], F32, tag="s1")
                    for k in range(KO):
                        nc.tensor.matmul(
                            ps[:],
                            wqk_sb[:, k, ts(m, P)],
                            xt[:, k, :],
                            start=(k == 0),
                            stop=(k == KO - 1),
                        )
                    if m < NH * HD // P:
                        nc.vector.tensor_scalar_add(
                            qt[:, m, :], ps[:], bqk_sb[:, m : m + 1]
                        )
                    else:
                        mk = m - NH * HD // P
                        hA, hB = 2 * mk, 2 * mk + 1
                        tsl = ts(tc_i, QC)
                        nc.vector.tensor_scalar_add(
                            kt_pad[0:HD, hA, tsl], ps[0:HD, :],
                            bqk_sb[0:HD, m : m + 1],
                        )
                        nc.vector.tensor_scalar_add(
                            kt_pad[HD:P, hB, tsl], ps[HD:P, :],
                            bqk_sb[HD:P, m : m + 1],
                        )

                # V rows for the 4 k-blocks of this T-chunk
                for t4 in range(QC // P):
                    kb = tc_i * (QC // P) + t4
                    psv = ps_s1.tile([P, NH * HD], F32, tag="s1")
                    for k in range(KO):
                        nc.tensor.matmul(
                            psv[:],
                            xt[:, k, ts(t4, P)],
                            wv_sb[:, k, :],
                            start=(k == 0),
                            stop=(k == KO - 1),
                        )
                    nc.vector.tensor_add(
                        vex[:, kb, :, :HD],
                        psv[:].rearrange("p (h d) -> p h d", h=NH),
                        bv_sb[:].rearrange("p (h d) -> p h d", h=NH),
                    )

                # ---------- attention + proj for q-chunk == this T-chunk ----------
                qc = tc_i
                nkb = (qc + 1) * (QC // P)
                ytq = ytpool.tile([P, NH * HD // P, QC], BF16)     # [128, 4, 512]
                for g in range(NH // 2):  # head pairs
                    yacs = []
                    for hh in range(2):
                        h = 2 * g + hh
                        yac = ps_y.tile([HD + 1, QC], F32)
                        yacs.append(yac)
                        for kb in range(nkb):
                            d = kb - qc * (QC // P)
                            off = 0 if d < 0 else d * P
                            w = QC - off
                            sps = ps_sc.tile([P, QC], F32, tag="sps")
                            nc.tensor.matmul(
                                sps[:, :w],
                                kt_pad[:, h, ts(kb, P)],
                                qt[:, g, off:QC],
                                start=True,
                                stop=True,
                            )
                            pt = ptpool.tile([P, QC], BF16)
                            nc.scalar.activation(
                                pt[:, :w], sps[:, :w],
                                mybir.ActivationFunctionType.Exp,
                                scale=1.0 / np.sqrt(HD),
                            )
                            if d >= 0:  # boundary 128 cols get the j>=p mask
                                nc.gpsimd.tensor_mul(
                                    pt[:, :P], pt[:, :P], mask_sb[:]
                                )
                            nc.tensor.matmul(
                                yac[:, off:QC],
                                vex[:, kb, h, :],
                                pt[:, :w],
                                start=(kb == 0),
                                stop=(kb == nkb - 1),
                            )
                    # normalize the pair: 1/denominator (one on DVE, one as
                    # exp(-ln) on ScalarE to split the load; Exp+Ln share an
                    # activation table so no table reloads), broadcast via
                    # one full 128-contraction matmul, then scale y
                    nc.vector.reciprocal(
                        reczero[0:1, :], yacs[0][HD : HD + 1, :]
                    )
                    lg = lnpool.tile([1, QC], F32)
                    nc.scalar.activation(
                        lg[:], yacs[1][HD : HD + 1, :],
                        mybir.ActivationFunctionType.Ln,
                    )
                    nc.scalar.activation(
                        reczero[HD : HD + 1, :], lg[:],
                        mybir.ActivationFunctionType.Exp, scale=-1.0,
                    )
                    recb = ps_sc.tile([P, QC], F32, tag="sps")
                    nc.tensor.matmul(
                        recb[:], ones2[:], reczero[:], start=True, stop=True
                    )
                    recb_sb = rbpool.tile([P, QC], BF16)
                    nc.vector.tensor_copy(recb_sb[:], recb[:])
                    nc.vector.tensor_mul(
                        ytq[0:HD, g, :], yacs[0][:HD, :], recb_sb[0:HD, :]
                    )
                    nc.vector.tensor_mul(
                        ytq[HD:P, g, :], yacs[1][:HD, :], recb_sb[HD:P, :]
                    )

                # proj: y_part^T[:, qc] = Wp_slice^T-contract @ ytq
                for m in range(C // P):  # 8
                    pp = ps_p.tile([P, QC], F32)
                    for kk in range(NH * HD // P):  # 4
                        nc.tensor.matmul(
                            pp[:],
                            wp_sb[:, kk, ts(m, P)],
                            ytq[:, kk, :],
                            start=(kk == 0),
                            stop=(kk == NH * HD // P - 1),
                        )
                    st = stpool.tile([P, QC], F32)
                    nc.vector.tensor_copy(st[:], pp[:])
                    nc.sync.dma_start(yT_t[:, m, ts(qc, QC)], st[:])

    return nc


def legalize_waits(nc):
    """This walrus build accepts at most 1 sync wait per instruction (0 for
    self-loading fp32/fp32r Matmult, whose LW slot takes none). Move excess
    waits onto preceding same-engine NoOps; engines execute in order so the
    guarantee is identical."""
    n = 0
    for blk in nc.m.functions[0].blocks:
        new = []
        for inst in blk.instructions:
            si = inst.sync_info
            waits = list(si.on_wait) if si is not None and si.on_wait else []
            lim = 0 if inst.opcode in ("Matmult", "Ldweights") else 1
            if len(waits) > lim:
                keep = waits[len(waits) - lim:] if lim else []
                for w in waits[: len(waits) - lim]:
                    n += 1
                    new.append(mybir.InstNoOp(
                        name=f"I-wfix{n}", engine=inst.engine, ins=[], outs=[],
                        sync_info=mybir.SyncInfo(on_wait=[w], on_update=[]),
                    ))
                inst.sync_info = mybir.SyncInfo(
                    on_wait=keep,
                    on_update=list(si.on_update) if si.on_update else [],
                )
            new.append(inst)
        blk.instructions = new
    return n


def _host_inputs(x, W_attn, b_attn, W_proj):
    """Build the 8 per-core input maps."""
    kl = np.arange(P)[:, None]
    ql = np.arange(P)[None, :]
    mask = (ql >= kl).astype(ml_dtypes.bfloat16)  # [128, 128]

    in_maps = []
    for core in range(8):
        b, g = core // 2, core % 2
        qs = slice(g * NH * HD, (g + 1) * NH * HD)
        ks = slice(C + g * NH * HD, C + (g + 1) * NH * HD)
        vs = slice(2 * C + g * NH * HD, 2 * C + (g + 1) * NH * HD)
        wqk = np.ascontiguousarray(
            np.concatenate([W_attn[:, qs], W_attn[:, ks]], axis=1)
        )
        bqk = (
            np.concatenate([b_attn[qs], b_attn[ks]])
            .reshape(2 * NH * HD // P, P)
            .T.copy()
        )
        in_maps.append(
            {
                "xT": np.ascontiguousarray(x[b].T),
                "Wqk": wqk,
                "Wv": np.ascontiguousarray(W_attn[:, vs]),
                "Wp": np.ascontiguousarray(
                    W_proj[g * NH * HD : (g + 1) * NH * HD]
                ).astype(ml_dtypes.bfloat16),
                "bqk": np.ascontiguousarray(bqk),
                "bv": np.ascontiguousarray(b_attn[vs]),
                "mask": mask,
            }
        )
    return in_maps


def run(x, W_attn, b_attn, W_proj, b_proj, trace=False):
    """Returns (y, BassKernelResults)."""
    x = np.asarray(x, dtype=np.float32)
    W_attn = np.asarray(W_attn, dtype=np.float32)
    b_attn = np.asarray(b_attn, dtype=np.float32)
    W_proj = np.asarray(W_proj, dtype=np.float32)
    b_proj = np.asarray(b_proj, dtype=np.float32)

    nc = build_nc()
    legalize_waits(nc)
    in_maps = _host_inputs(x, W_attn, b_attn, W_proj)
    res = run_bass_kernel_spmd(nc, in_maps, list(range(8)), trace=trace)

    y = np.empty((B, T, C), dtype=np.float32)
    for b in range(B):
        acc = res.results[2 * b]["yT"] + res.results[2 * b + 1]["yT"]
        y[b] = acc.T + b_proj
    return y, res


def kernel(x, W_attn, b_attn, W_proj, b_proj):
    y, _ = run(x, W_attn, b_attn, W_proj, b_proj)
    return y


# revision 15
# speedup vs baseline: 1.7824x; 1.0207x over previous
"""Causal self-attention (B=4, T=2048, C=1024, H=16) on 8 TRN2 NeuronCores.

Sharding: core c -> (batch b = c//2, head-group g = c%2). Each core computes
QKV for its 8 heads of one batch, causal attention, and a partial output
projection (its heads' slice of W_proj). The pairwise reduction over head
groups (the "all-reduce after c_proj") plus b_proj is done on host at
gather time.

On-core layout (v2 — PE kept at full clock):
  xT        [C, T]     (host pre-transposes x[b]); stage-1 QKV in fp32r
  qt        [128, 4, 512] bf16 — Q^T per chunk, head pair on partitions
  kt_pad    [128, 8, T] bf16 — per-head K^T, zero-padded to the full 128
            contraction partitions so S matmuls are full unmasked
            128x128xW (row_grp-masked matmuls don't register as PE
            activity in the HAM clock gate and throttle the PE to 1.2GHz)
  S         [128 k-rows, W q] psum; W trimmed to the causal width
            (512-128d on diagonal blocks); exp on ScalarE -> pt bf16;
            the single 128x128 (j>=p) boundary mask applied on GpSimd
  y         [65, 512] psum per (head, chunk): rows 0:64 = V^T P, row 64 =
            softmax denominator via a bf16 ones column in vex
  recip     reciprocal_approx_fast (DVE custom op, ~5x cheaper), then one
            full 128-contraction broadcast matmul per head PAIR
  proj      Wp bf16; psum -> SBUF copy -> DMA out as y_part^T [C, T];
            host transposes + sums the two head groups + b_proj.
"""

import sys

for _p in ("/opt/trn_rl_repo", "/root/.axon_site/_ro/trn_rl_repo"):
    if _p not in sys.path:
        sys.path.insert(0, _p)

import ml_dtypes
import numpy as np

import concourse.bass as bass
import concourse.mybir as mybir
import concourse.tile as tile
from concourse.bass import ts
from concourse.bass_utils import run_bass_kernel_spmd

B, T, C, H, HD = 4, 2048, 1024, 16, 64
NH = 8           # heads per core
P = 128
QC = 512         # q-chunk width
NQC = T // QC    # 4
NKB = T // P     # 16 k-blocks
KO = C // P      # 8 contraction tiles for the C-dim
F32 = mybir.dt.float32
F32R = mybir.dt.float32r
BF16 = mybir.dt.bfloat16


def build_nc():
    nc = bass.Bass()

    xT = nc.dram_tensor("xT", [C, T], BF16, kind="ExternalInput")
    Wqk = nc.dram_tensor("Wqk", [C, 2 * NH * HD], BF16, kind="ExternalInput")
    Wv = nc.dram_tensor("Wv", [C, NH * HD], BF16, kind="ExternalInput")
    Wp = nc.dram_tensor("Wp", [NH * HD, C], BF16, kind="ExternalInput")
    bqk = nc.dram_tensor("bqk", [P, 2 * NH * HD // P], F32, kind="ExternalInput")
    bv = nc.dram_tensor("bv", [NH * HD], F32, kind="ExternalInput")
    mask = nc.dram_tensor("mask", [P, P], BF16, kind="ExternalInput")
    yT = nc.dram_tensor("yT", [C, T], F32, kind="ExternalOutput")

    xT_t = xT[:].rearrange("(ko p) t -> p ko t", p=P)        # [128, 8, T]
    yT_t = yT[:].rearrange("(mo p) t -> p mo t", p=P)        # [128, 8, T]
    Wqk_t = Wqk[:].rearrange("(ko p) n -> p ko n", p=P)      # [128, 8, 1024]
    Wv_t = Wv[:].rearrange("(ko p) n -> p ko n", p=P)        # [128, 8, 512]
    Wp_t = Wp[:].rearrange("(ko p) n -> p ko n", p=P)        # [128, 4, 1024]

    with tile.TileContext(nc) as tc:
        with (
            tc.tile_pool(name="consts", bufs=1) as consts,
            tc.tile_pool(name="persist", bufs=1) as persist,
            tc.tile_pool(name="w1", bufs=1) as w1pool,
            tc.tile_pool(name="xt", bufs=2) as xtpool,
            tc.tile_pool(name="qt", bufs=2) as qtpool,
            tc.tile_pool(name="yt", bufs=2) as ytpool,
            tc.tile_pool(name="pt", bufs=3) as ptpool,
            tc.tile_pool(name="recb", bufs=2) as rbpool,
            tc.tile_pool(name="ln", bufs=2) as lnpool,
            tc.tile_pool(name="st", bufs=2) as stpool,
            tc.tile_pool(name="ps_s1", bufs=2, space="PSUM") as ps_s1,
            tc.tile_pool(name="ps_sc", bufs=2, space="PSUM") as ps_sc,
            tc.tile_pool(name="ps_y", bufs=2, space="PSUM") as ps_y,
            tc.tile_pool(name="ps_p", bufs=2, space="PSUM") as ps_p,
        ):
            # ---- constants ----
            bqk_sb = consts.tile([P, 2 * NH * HD // P], F32)      # [128, 8]
            nc.sync.dma_start(bqk_sb[:], bqk[:])
            bv_sb = consts.tile([P, NH * HD], F32)                # [128, 512]
            nc.sync.dma_start(bv_sb[:], bass.AP(bv, 0, [[0, P], [1, NH * HD]]))
            mask_sb = consts.tile([P, P], BF16)                    # j >= p
            nc.sync.dma_start(mask_sb[:], mask[:])

            # recb broadcast operands: ones2 row 0 selects out cols 0:64
            # (head A), row 64 cols 64:128 (head B); other rows are zero, as
            # are reczero's other moving rows, so the matmul only sees the
            # two recs. (Row 64, not 1: engine APs need base partition
            # 0/32/64/96.)
            ones2 = consts.tile([P, P], F32)
            nc.vector.memset(ones2[:], 0.0)
            nc.vector.memset(ones2[0:1, 0:HD], 1.0)
            nc.vector.memset(ones2[HD : HD + 1, HD : 2 * HD], 1.0)
            reczero = consts.tile([P, QC], F32)
            nc.vector.memset(reczero[:], 0.0)

            # ---- persistent activations ----
            kt_pad = persist.tile([P, NH, T], BF16)                # 4MB
            nc.vector.memset(kt_pad[HD:P, 0:NH:2, :], 0.0)
            nc.vector.memset(kt_pad[0:HD, 1:NH:2, :], 0.0)
            vex = persist.tile([P, NKB, NH, HD + 1], BF16)         # 2.1MB
            nc.vector.memset(vex[:, :, :, HD:], 1.0)

            # ---- stage-1 weights (split per k-tile so the first m-loop
            # matmul only gates on slice 0, not the whole tensor) ----
            wqk_sb = w1pool.tile([P, KO, 2 * NH * HD], BF16)       # 2MB
            for k in range(KO):
                nc.sync.dma_start(wqk_sb[:, k, :], Wqk_t[:, k, :])
            wv_sb = w1pool.tile([P, KO, NH * HD], BF16)            # 1MB
            for k in range(KO):
                nc.sync.dma_start(wv_sb[:, k, :], Wv_t[:, k, :])
            # Wp is only needed at the first proj (~90us in): load it last
            wp_sb = consts.tile([P, NH * HD // P, C], BF16)        # [128, 4, 1024]
            nc.sync.dma_start(wp_sb[:], Wp_t[:])

            for tc_i in range(NQC):  # T chunk of 512
                # ---------- stage 1 for this T-chunk ----------
                # x arrives per k-tile on the GpSimd DMA queue so it isn't
                # serialized behind the weight stream on Sync
                xt = xtpool.tile([P, KO, QC], BF16)
                for k in range(KO):
                    nc.gpsimd.dma_start(xt[:, k, :], xT_t[:, k, ts(tc_i, QC)])

                qt = qtpool.tile([P, NH * HD // P, QC], BF16)
                for m in range(2 * NH * HD // P):  # 8: m<4 Q, m>=4 K
                    ps = ps_s1.tile([P, QC], F32, tag="s1")
                    for k in range(KO):
                        nc.tensor.matmul(
                            ps[:],
                            wqk_sb[:, k, ts(m, P)],
                            xt[:, k, :],
                            start=(k == 0),
                            stop=(k == KO - 1),
                        )
                    if m < NH * HD // P:
                        nc.vector.tensor_scalar_add(
                            qt[:, m, :], ps[:], bqk_sb[:, m : m + 1]
                        )
                    else:
                        mk = m - NH * HD // P
                        hA, hB = 2 * mk, 2 * mk + 1
                        tsl = ts(tc_i, QC)
                        nc.vector.tensor_scalar_add(
                            kt_pad[0:HD, hA, tsl], ps[0:HD, :],
                            bqk_sb[0:HD, m : m + 1],
                        )
                        nc.vector.tensor_scalar_add(
                            kt_pad[HD:P, hB, tsl], ps[HD:P, :],
                            bqk_sb[HD:P, m : m + 1],
                        )

                # V rows for the 4 k-blocks of this T-chunk
                for t4 in range(QC // P):
                    kb = tc_i * (QC // P) + t4
                    psv = ps_s1.tile([P, NH * HD], F32, tag="s1")
                    for k in range(KO):
                        nc.tensor.matmul(
                            psv[:],
                            xt[:, k, ts(t4, P)],
                            wv_sb[:, k, :],
                            start=(k == 0),
                            stop=(k == KO - 1),
                        )
                    nc.vector.tensor_add(
                        vex[:, kb, :, :HD],
                        psv[:].rearrange("p (h d) -> p h d", h=NH),
                        bv_sb[:].rearrange("p (h d) -> p h d", h=NH),
                    )

                # ---------- attention + proj for q-chunk == this T-chunk ----------
                qc = tc_i
                nkb = (qc + 1) * (QC // P)
                ytq = ytpool.tile([P, NH * HD // P, QC], BF16)     # [128, 4, 512]
                for g in range(NH // 2):  # head pairs
                    yacs = []
                    for hh in range(2):
                        h = 2 * g + hh
                        yac = ps_y.tile([HD + 1, QC], F32)
                        yacs.append(yac)
                        for kb in range(nkb):
                            d = kb - qc * (QC // P)
                            off = 0 if d < 0 else d * P
                            w = QC - off
                            sps = ps_sc.tile([P, QC], F32, tag="sps")
                            nc.tensor.matmul(
                                sps[:, :w],
                                kt_pad[:, h, ts(kb, P)],
                                qt[:, g, off:QC],
                                start=True,
                                stop=True,
                            )
                            pt = ptpool.tile([P, QC], BF16)
                            nc.scalar.activation(
                                pt[:, :w], sps[:, :w],
                                mybir.ActivationFunctionType.Exp,
                                scale=1.0 / np.sqrt(HD),
                            )
                            if d >= 0:  # boundary 128 cols get the j>=p mask
                                nc.gpsimd.tensor_mul(
                                    pt[:, :P], pt[:, :P], mask_sb[:]
                                )
                            nc.tensor.matmul(
                                yac[:, off:QC],
                                vex[:, kb, h, :],
                                pt[:, :w],
                                start=(kb == 0),
                                stop=(kb == nkb - 1),
                            )
                    # normalize the pair: 1/denominator (one on DVE, one as
                    # exp(-ln) on ScalarE to split the load; Exp+Ln share an
                    # activation table so no table reloads), broadcast via
                    # one full 128-contraction matmul, then scale y
                    nc.vector.reciprocal(
                        reczero[0:1, :], yacs[0][HD : HD + 1, :]
                    )
                    lg = lnpool.tile([1, QC], F32)
                    nc.scalar.activation(
                        lg[:], yacs[1][HD : HD + 1, :],
                        mybir.ActivationFunctionType.Ln,
                    )
                    nc.scalar.activation(
                        reczero[HD : HD + 1, :], lg[:],
                        mybir.ActivationFunctionType.Exp, scale=-1.0,
                    )
                    recb = ps_sc.tile([P, QC], F32, tag="sps")
                    nc.tensor.matmul(
                        recb[:], ones2[:], reczero[:], start=True, stop=True
                    )
                    recb_sb = rbpool.tile([P, QC], BF16)
                    nc.vector.tensor_copy(recb_sb[:], recb[:])
                    nc.vector.tensor_mul(
                        ytq[0:HD, g, :], yacs[0][:HD, :], recb_sb[0:HD, :]
                    )
                    nc.vector.tensor_mul(
                        ytq[HD:P, g, :], yacs[1][:HD, :], recb_sb[HD:P, :]
                    )

                # proj: y_part^T[:, qc] = Wp_slice^T-contract @ ytq
                for m in range(C // P):  # 8
                    pp = ps_p.tile([P, QC], F32)
                    for kk in range(NH * HD // P):  # 4
                        nc.tensor.matmul(
                            pp[:],
                            wp_sb[:, kk, ts(m, P)],
                            ytq[:, kk, :],
                            start=(kk == 0),
                            stop=(kk == NH * HD // P - 1),
                        )
                    st = stpool.tile([P, QC], F32)
                    nc.vector.tensor_copy(st[:], pp[:])
                    nc.sync.dma_start(yT_t[:, m, ts(qc, QC)], st[:])

    return nc


def legalize_waits(nc):
    """This walrus build accepts at most 1 sync wait per instruction (0 for
    self-loading fp32/fp32r Matmult, whose LW slot takes none). Move excess
    waits onto preceding same-engine NoOps; engines execute in order so the
    guarantee is identical."""
    n = 0
    for blk in nc.m.functions[0].blocks:
        new = []
        for inst in blk.instructions:
            si = inst.sync_info
            waits = list(si.on_wait) if si is not None and si.on_wait else []
            lim = 0 if inst.opcode in ("Matmult", "Ldweights") else 1
            if len(waits) > lim:
                keep = waits[len(waits) - lim:] if lim else []
                for w in waits[: len(waits) - lim]:
                    n += 1
                    new.append(mybir.InstNoOp(
                        name=f"I-wfix{n}", engine=inst.engine, ins=[], outs=[],
                        sync_info=mybir.SyncInfo(on_wait=[w], on_update=[]),
                    ))
                inst.sync_info = mybir.SyncInfo(
                    on_wait=keep,
                    on_update=list(si.on_update) if si.on_update else [],
                )
            new.append(inst)
        blk.instructions = new
    return n


def _host_inputs(x, W_attn, b_attn, W_proj):
    """Build the 8 per-core input maps."""
    kl = np.arange(P)[:, None]
    ql = np.arange(P)[None, :]
    mask = (ql >= kl).astype(ml_dtypes.bfloat16)  # [128, 128]

    in_maps = []
    for core in range(8):
        b, g = core // 2, core % 2
        qs = slice(g * NH * HD, (g + 1) * NH * HD)
        ks = slice(C + g * NH * HD, C + (g + 1) * NH * HD)
        vs = slice(2 * C + g * NH * HD, 2 * C + (g + 1) * NH * HD)
        wqk = np.ascontiguousarray(
            np.concatenate([W_attn[:, qs], W_attn[:, ks]], axis=1)
        )
        bqk = (
            np.concatenate([b_attn[qs], b_attn[ks]])
            .reshape(2 * NH * HD // P, P)
            .T.copy()
        )
        in_maps.append(
            {
                "xT": np.ascontiguousarray(x[b].T).astype(ml_dtypes.bfloat16),
                "Wqk": wqk.astype(ml_dtypes.bfloat16),
                "Wv": np.ascontiguousarray(W_attn[:, vs]).astype(
                    ml_dtypes.bfloat16
                ),
                "Wp": np.ascontiguousarray(
                    W_proj[g * NH * HD : (g + 1) * NH * HD]
                ).astype(ml_dtypes.bfloat16),
                "bqk": np.ascontiguousarray(bqk),
                "bv": np.ascontiguousarray(b_attn[vs]),
                "mask": mask,
            }
        )
    return in_maps


def run(x, W_attn, b_attn, W_proj, b_proj, trace=False):
    """Returns (y, BassKernelResults)."""
    x = np.asarray(x, dtype=np.float32)
    W_attn = np.asarray(W_attn, dtype=np.float32)
    b_attn = np.asarray(b_attn, dtype=np.float32)
    W_proj = np.asarray(W_proj, dtype=np.float32)
    b_proj = np.asarray(b_proj, dtype=np.float32)

    nc = build_nc()
    legalize_waits(nc)
    in_maps = _host_inputs(x, W_attn, b_attn, W_proj)
    res = run_bass_kernel_spmd(nc, in_maps, list(range(8)), trace=trace)

    y = np.empty((B, T, C), dtype=np.float32)
    for b in range(B):
        acc = res.results[2 * b]["yT"] + res.results[2 * b + 1]["yT"]
        y[b] = acc.T + b_proj
    return y, res


def kernel(x, W_attn, b_attn, W_proj, b_proj):
    y, _ = run(x, W_attn, b_attn, W_proj, b_proj)
    return y


# revision 20
# speedup vs baseline: 1.8644x; 1.0460x over previous
"""Causal self-attention (B=4, T=2048, C=1024, H=16) on 8 TRN2 NeuronCores.

Sharding: core c -> (batch b = c//2, head-group g = c%2). Each core computes
QKV for its 8 heads of one batch, causal attention, and a partial output
projection (its heads' slice of W_proj). The pairwise reduction over head
groups (the "all-reduce after c_proj") plus b_proj is done on host at
gather time.

On-core layout (v2 — PE kept at full clock):
  xT        [C, T]     (host pre-transposes x[b]); stage-1 QKV in fp32r
  qt        [128, 4, 512] bf16 — Q^T per chunk, head pair on partitions
  kt_pad    [128, 8, T] bf16 — per-head K^T, zero-padded to the full 128
            contraction partitions so S matmuls are full unmasked
            128x128xW (row_grp-masked matmuls don't register as PE
            activity in the HAM clock gate and throttle the PE to 1.2GHz)
  S         [128 k-rows, W q] psum; W trimmed to the causal width
            (512-128d on diagonal blocks); exp on ScalarE -> pt bf16;
            the single 128x128 (j>=p) boundary mask applied on GpSimd
  y         [65, 512] psum per (head, chunk): rows 0:64 = V^T P, row 64 =
            softmax denominator via a bf16 ones column in vex
  recip     reciprocal_approx_fast (DVE custom op, ~5x cheaper), then one
            full 128-contraction broadcast matmul per head PAIR
  proj      Wp bf16; psum -> SBUF copy -> DMA out as y_part^T [C, T];
            host transposes + sums the two head groups + b_proj.
"""

import sys

for _p in ("/opt/trn_rl_repo", "/root/.axon_site/_ro/trn_rl_repo"):
    if _p not in sys.path:
        sys.path.insert(0, _p)

import ml_dtypes
import numpy as np

import concourse.bass as bass
import concourse.mybir as mybir
import concourse.tile as tile
from concourse.bass import ts
from concourse.bass_utils import run_bass_kernel_spmd

B, T, C, H, HD = 4, 2048, 1024, 16, 64
NH = 8           # heads per core
P = 128
QC = 512         # q-chunk width
NQC = T // QC    # 4
NKB = T // P     # 16 k-blocks
KO = C // P      # 8 contraction tiles for the C-dim
F32 = mybir.dt.float32
F32R = mybir.dt.float32r
BF16 = mybir.dt.bfloat16


def build_nc():
    nc = bass.Bass()

    xT = nc.dram_tensor("xT", [C, T], BF16, kind="ExternalInput")
    Wqk = nc.dram_tensor("Wqk", [C, 2 * NH * HD], BF16, kind="ExternalInput")
    Wv = nc.dram_tensor("Wv", [C, NH * HD], BF16, kind="ExternalInput")
    Wp = nc.dram_tensor("Wp", [NH * HD, C], BF16, kind="ExternalInput")
    bqk = nc.dram_tensor("bqk", [P, 2 * NH * HD // P], F32, kind="ExternalInput")
    bv = nc.dram_tensor("bv", [NH * HD], F32, kind="ExternalInput")
    mask = nc.dram_tensor("mask", [P, P], BF16, kind="ExternalInput")
    yT = nc.dram_tensor("yT", [C, T], F32, kind="ExternalOutput")

    xT_t = xT[:].rearrange("(ko p) t -> p ko t", p=P)        # [128, 8, T]
    yT_t = yT[:].rearrange("(mo p) t -> p mo t", p=P)        # [128, 8, T]
    Wqk_t = Wqk[:].rearrange("(ko p) n -> p ko n", p=P)      # [128, 8, 1024]
    Wv_t = Wv[:].rearrange("(ko p) n -> p ko n", p=P)        # [128, 8, 512]
    Wp_t = Wp[:].rearrange("(ko p) n -> p ko n", p=P)        # [128, 4, 1024]

    with tile.TileContext(nc) as tc:
        with (
            tc.tile_pool(name="consts", bufs=1) as consts,
            tc.tile_pool(name="persist", bufs=1) as persist,
            tc.tile_pool(name="w1", bufs=1) as w1pool,
            tc.tile_pool(name="xt", bufs=2) as xtpool,
            tc.tile_pool(name="qt", bufs=2) as qtpool,
            tc.tile_pool(name="yt", bufs=2) as ytpool,
            tc.tile_pool(name="pt", bufs=3) as ptpool,
            tc.tile_pool(name="recb", bufs=2) as rbpool,
            tc.tile_pool(name="ln", bufs=2) as lnpool,
            tc.tile_pool(name="st", bufs=2) as stpool,
            tc.tile_pool(name="ps_s1", bufs=2, space="PSUM") as ps_s1,
            tc.tile_pool(name="ps_sc", bufs=2, space="PSUM") as ps_sc,
            tc.tile_pool(name="ps_y", bufs=2, space="PSUM") as ps_y,
            tc.tile_pool(name="ps_p", bufs=2, space="PSUM") as ps_p,
        ):
            # ---- constants ----
            bqk_sb = consts.tile([P, 2 * NH * HD // P], F32)      # [128, 8]
            nc.sync.dma_start(bqk_sb[:], bqk[:])
            bv_sb = consts.tile([P, NH * HD], F32)                # [128, 512]
            nc.sync.dma_start(bv_sb[:], bass.AP(bv, 0, [[0, P], [1, NH * HD]]))
            mask_sb = consts.tile([P, P], BF16)                    # j >= p
            nc.sync.dma_start(mask_sb[:], mask[:])

            # recb broadcast operands: ones2 row 0 selects out cols 0:64
            # (head A), row 64 cols 64:128 (head B); other rows are zero, as
            # are the rz tiles' other moving rows, so the matmul only sees
            # the two reciprocal rows. (Partition bases must be 0/32/64/96.)
            ones2 = consts.tile([P, P], F32)
            nc.vector.memset(ones2[:], 0.0)
            nc.vector.memset(ones2[0:1, 0:HD], 1.0)
            nc.vector.memset(ones2[HD : HD + 1, HD : 2 * HD], 1.0)
            # two alternating reciprocal-row tiles (double-buffered by pair
            # parity so a deferred recb matmul never races the next pair)
            rzs = []
            for _ in range(2):
                rz = consts.tile([P, QC], F32, tag=f"rz{_}")
                nc.vector.memset(rz[:], 0.0)
                rzs.append(rz)

            # ---- persistent activations ----
            kt_pad = persist.tile([P, NH, T], BF16)                # 4MB
            nc.vector.memset(kt_pad[HD:P, 0:NH:2, :], 0.0)
            nc.vector.memset(kt_pad[0:HD, 1:NH:2, :], 0.0)
            vex = persist.tile([P, NKB, NH, HD + 1], BF16)         # 2.1MB
            nc.vector.memset(vex[:, :, :, HD:], 1.0)

            # ---- stage-1 weights (split per k-tile so the first m-loop
            # matmul only gates on slice 0, not the whole tensor) ----
            wqk_sb = w1pool.tile([P, KO, 2 * NH * HD], BF16)       # 2MB
            for k in range(KO):
                nc.sync.dma_start(wqk_sb[:, k, :], Wqk_t[:, k, :])
            wv_sb = w1pool.tile([P, KO, NH * HD], BF16)            # 1MB
            for k in range(KO):
                nc.sync.dma_start(wv_sb[:, k, :], Wv_t[:, k, :])
            # Wp is only needed at the first proj (~90us in): load it last
            wp_sb = consts.tile([P, NH * HD // P, C], BF16)        # [128, 4, 1024]
            nc.sync.dma_start(wp_sb[:], Wp_t[:])

            for tc_i in range(NQC):  # T chunk of 512
                # ---------- stage 1 for this T-chunk ----------
                # x arrives per k-tile on the GpSimd DMA queue so it isn't
                # serialized behind the weight stream on Sync
                xt = xtpool.tile([P, KO, QC], BF16)
                for k in range(KO):
                    nc.gpsimd.dma_start(xt[:, k, :], xT_t[:, k, ts(tc_i, QC)])

                qt = qtpool.tile([P, NH * HD // P, QC], BF16)
                for m in range(2 * NH * HD // P):  # 8: m<4 Q, m>=4 K
                    ps = ps_s1.tile([P, QC], F32, tag="s1")
                    for k in range(KO):
                        nc.tensor.matmul(
                            ps[:],
                            wqk_sb[:, k, ts(m, P)],
                            xt[:, k, :],
                            start=(k == 0),
                            stop=(k == KO - 1),
                        )
                    if m < NH * HD // P:
                        nc.vector.tensor_scalar_add(
                            qt[:, m, :], ps[:], bqk_sb[:, m : m + 1]
                        )
                    else:
                        mk = m - NH * HD // P
                        hA, hB = 2 * mk, 2 * mk + 1
                        tsl = ts(tc_i, QC)
                        nc.vector.tensor_scalar_add(
                            kt_pad[0:HD, hA, tsl], ps[0:HD, :],
                            bqk_sb[0:HD, m : m + 1],
                        )
                        nc.vector.tensor_scalar_add(
                            kt_pad[HD:P, hB, tsl], ps[HD:P, :],
                            bqk_sb[HD:P, m : m + 1],
                        )

                # V rows for the 4 k-blocks of this T-chunk
                for t4 in range(QC // P):
                    kb = tc_i * (QC // P) + t4
                    psv = ps_s1.tile([P, NH * HD], F32, tag="s1")
                    for k in range(KO):
                        nc.tensor.matmul(
                            psv[:],
                            xt[:, k, ts(t4, P)],
                            wv_sb[:, k, :],
                            start=(k == 0),
                            stop=(k == KO - 1),
                        )
                    nc.vector.tensor_add(
                        vex[:, kb, :, :HD],
                        psv[:].rearrange("p (h d) -> p h d", h=NH),
                        bv_sb[:].rearrange("p (h d) -> p h d", h=NH),
                    )

                # ---------- attention + proj for q-chunk == this T-chunk ----------
                qc = tc_i
                nkb = (qc + 1) * (QC // P)
                ytq = ytpool.tile([P, NH * HD // P, QC], BF16)     # [128, 4, 512]

                def emit_norm(g, yu, rz):
                    # 1/denominator rows were computed right after pair g's
                    # attention; this broadcast matmul + scale is emitted one
                    # pair late so the in-order PE queue never waits on the
                    # slow reciprocal.
                    recb = ps_sc.tile([P, QC], F32, tag="sps")
                    nc.tensor.matmul(
                        recb[:], ones2[:], rz[:], start=True, stop=True
                    )
                    nc.vector.tensor_mul(
                        ytq[0:HD, g, :], yu[0:HD, :], recb[0:HD, :]
                    )
                    nc.vector.tensor_mul(
                        ytq[HD:P, g, :], yu[HD:P, :], recb[HD:P, :]
                    )

                pending = None
                for g in range(NH // 2):  # head pairs
                    yacs = []
                    for hh in range(2):
                        h = 2 * g + hh
                        yac = ps_y.tile([HD + 1, QC], F32)
                        yacs.append(yac)
                        for kb in range(nkb):
                            d = kb - qc * (QC // P)
                            off = 0 if d < 0 else d * P
                            w = QC - off
                            sps = ps_sc.tile([P, QC], F32, tag="sps")
                            nc.tensor.matmul(
                                sps[:, :w],
                                kt_pad[:, h, ts(kb, P)],
                                qt[:, g, off:QC],
                                start=True,
                                stop=True,
                            )
                            pt = ptpool.tile([P, QC], BF16)
                            nc.scalar.activation(
                                pt[:, :w], sps[:, :w],
                                mybir.ActivationFunctionType.Exp,
                                scale=1.0 / np.sqrt(HD),
                            )
                            if d >= 0:  # boundary 128 cols get the j>=p mask
                                nc.gpsimd.tensor_mul(
                                    pt[:, :P], pt[:, :P], mask_sb[:]
                                )
                            nc.tensor.matmul(
                                yac[:, off:QC],
                                vex[:, kb, h, :],
                                pt[:, :w],
                                start=(kb == 0),
                                stop=(kb == nkb - 1),
                            )
                    # stage unnormalized y to SBUF (frees the yac banks for
                    # the next pair) and kick off the pair's reciprocals:
                    # one on DVE, one as exp(-ln) on ScalarE (Exp+Ln share an
                    # activation table so no table reloads)
                    yu = rbpool.tile([P, QC], BF16)
                    nc.vector.tensor_copy(yu[0:HD, :], yacs[0][:HD, :])
                    nc.vector.tensor_copy(yu[HD:P, :], yacs[1][:HD, :])
                    rz = rzs[g % 2]
                    nc.vector.reciprocal(
                        rz[0:1, :], yacs[0][HD : HD + 1, :]
                    )
                    lg = lnpool.tile([1, QC], F32)
                    nc.scalar.activation(
                        lg[:], yacs[1][HD : HD + 1, :],
                        mybir.ActivationFunctionType.Ln,
                    )
                    nc.scalar.activation(
                        rz[HD : HD + 1, :], lg[:],
                        mybir.ActivationFunctionType.Exp, scale=-1.0,
                    )
                    if pending is not None:
                        emit_norm(*pending)
                    pending = (g, yu, rz)
                emit_norm(*pending)

                # proj: y_part^T[:, qc] = Wp_slice^T-contract @ ytq
                for m in range(C // P):  # 8
                    pp = ps_p.tile([P, QC], F32)
                    for kk in range(NH * HD // P):  # 4
                        nc.tensor.matmul(
                            pp[:],
                            wp_sb[:, kk, ts(m, P)],
                            ytq[:, kk, :],
                            start=(kk == 0),
                            stop=(kk == NH * HD // P - 1),
                        )
                    st = stpool.tile([P, QC], F32)
                    nc.vector.tensor_copy(st[:], pp[:])
                    nc.sync.dma_start(yT_t[:, m, ts(qc, QC)], st[:])

    return nc


def legalize_waits(nc):
    """This walrus build accepts at most 1 sync wait per instruction (0 for
    self-loading fp32/fp32r Matmult, whose LW slot takes none). Move excess
    waits onto preceding same-engine NoOps; engines execute in order so the
    guarantee is identical."""
    n = 0
    for blk in nc.m.functions[0].blocks:
        new = []
        for inst in blk.instructions:
            si = inst.sync_info
            waits = list(si.on_wait) if si is not None and si.on_wait else []
            lim = 0 if inst.opcode in ("Matmult", "Ldweights") else 1
            if len(waits) > lim:
                keep = waits[len(waits) - lim:] if lim else []
                for w in waits[: len(waits) - lim]:
                    n += 1
                    new.append(mybir.InstNoOp(
                        name=f"I-wfix{n}", engine=inst.engine, ins=[], outs=[],
                        sync_info=mybir.SyncInfo(on_wait=[w], on_update=[]),
                    ))
                inst.sync_info = mybir.SyncInfo(
                    on_wait=keep,
                    on_update=list(si.on_update) if si.on_update else [],
                )
            new.append(inst)
        blk.instructions = new
    return n


def _host_inputs(x, W_attn, b_attn, W_proj):
    """Build the 8 per-core input maps."""
    kl = np.arange(P)[:, None]
    ql = np.arange(P)[None, :]
    mask = (ql >= kl).astype(ml_dtypes.bfloat16)  # [128, 128]

    in_maps = []
    for core in range(8):
        b, g = core // 2, core % 2
        qs = slice(g * NH * HD, (g + 1) * NH * HD)
        ks = slice(C + g * NH * HD, C + (g + 1) * NH * HD)
        vs = slice(2 * C + g * NH * HD, 2 * C + (g + 1) * NH * HD)
        wqk = np.ascontiguousarray(
            np.concatenate([W_attn[:, qs], W_attn[:, ks]], axis=1)
        )
        bqk = (
            np.concatenate([b_attn[qs], b_attn[ks]])
            .reshape(2 * NH * HD // P, P)
            .T.copy()
        )
        in_maps.append(
            {
                "xT": np.ascontiguousarray(x[b].T).astype(ml_dtypes.bfloat16),
                "Wqk": wqk.astype(ml_dtypes.bfloat16),
                "Wv": np.ascontiguousarray(W_attn[:, vs]).astype(
                    ml_dtypes.bfloat16
                ),
                "Wp": np.ascontiguousarray(
                    W_proj[g * NH * HD : (g + 1) * NH * HD]
                ).astype(ml_dtypes.bfloat16),
                "bqk": np.ascontiguousarray(bqk),
                "bv": np.ascontiguousarray(b_attn[vs]),
                "mask": mask,
            }
        )
    return in_maps


def run(x, W_attn, b_attn, W_proj, b_proj, trace=False):
    """Returns (y, BassKernelResults)."""
    x = np.asarray(x, dtype=np.float32)
    W_attn = np.asarray(W_attn, dtype=np.float32)
    b_attn = np.asarray(b_attn, dtype=np.float32)
    W_proj = np.asarray(W_proj, dtype=np.float32)
    b_proj = np.asarray(b_proj, dtype=np.float32)

    nc = build_nc()
    legalize_waits(nc)
    in_maps = _host_inputs(x, W_attn, b_attn, W_proj)
    res = run_bass_kernel_spmd(nc, in_maps, list(range(8)), trace=trace)

    y = np.empty((B, T, C), dtype=np.float32)
    for b in range(B):
        acc = res.results[2 * b]["yT"] + res.results[2 * b + 1]["yT"]
        y[b] = acc.T + b_proj
    return y, res


def kernel(x, W_attn, b_attn, W_proj, b_proj):
    y, _ = run(x, W_attn, b_attn, W_proj, b_proj)
    return y
